# revision 22
# baseline (speedup 1.0000x reference)
"""ArcFace (AngularPenaltySMLoss) on 8 TRN2 NeuronCores.

Strategy (model-parallel softmax sharding):
  - Shard the 32768 classes across 8 cores (4096 classes each).
  - Host prep (layout only): transpose features -> fT [512, 2048] bf16,
    transpose each weight shard -> wT [512, 4096] bf16, gather target rows
    wtgt = weight[y_true] [2048, 512] f32.
  - Device, per core:
      * weight-col norms: squares (DVE) + ones-matmul partition-sum (PE),
        ACT Sqrt + DVE reciprocal_approx on rows, broadcast along partitions
        with a K=1 bf16 matmul; what = wT * bcast  [bf16, per 512-col chunk]
      * feature norms likewise, with the 1/4096 fold so the row already
        carries the ArcFace scale: fhat = 64 * normalized fT  [bf16]
      * main loop: z = fhat.T @ what accumulated over K=512 in PSUM (bf16
        matmuls); ACT Exp in place on PSUM with accum_out -> per-row partial
        exp sums (the full exp matrix is never stored)
      * target path (concurrent with main loop, on GpSimd+DVE): rawdot,
        ||f||^2, ||wtgt||^2 via gpsimd mult + DVE reduce (f32 exact)
      * the per-row exp sums AllReduce in TWO halves: the first half's
        AllReduce hides under the second half of the main loop
      * combine (ACT ops dep-gated behind the last main-loop Exp so the
        activation table isn't thrashed mid-loop):
        tgt = rawdot * exp(-0.5*ln(ssf*wn2));
        num = 64*(t*cos(m) - sqrt(1-t^2)*sin(m)) with sqrt via exp/ln;
        loss = -mean(num - ln(exp(num) + fullsum - exp(64*tgt)))
"""
import math

import numpy as np
import ml_dtypes

import concourse.bass as bass
import concourse.tile as tile
from concourse import bacc, mybir
from concourse.bass_utils import run_bass_kernel_spmd
from concourse.tile import add_dep_helper

B = 2048          # batch
D = 512           # feature dim
C = 32768         # classes
NCORES = 8
CS = C // NCORES  # 4096 classes per core
S = 64.0
MARGIN = 0.5
EPS = 1e-7
COSM = math.cos(MARGIN)
SINM = math.sin(MARGIN)

NB = B // 128     # 16 batch tiles
NK = D // 128     # 4 contraction chunks
NCC = CS // 512   # 8 class chunks per core
NBC = B // 512    # 4 batch chunks (row-layout ops)

F32 = mybir.dt.float32
BF16 = mybir.dt.bfloat16
AF = mybir.ActivationFunctionType
ALU = mybir.AluOpType
BF16NP = ml_dtypes.bfloat16
FP8 = mybir.dt.float8e4
FP8NP = ml_dtypes.float8_e4m3fn

USE_FP8 = True
MMDT = FP8 if USE_FP8 else BF16
MMNP = FP8NP if USE_FP8 else BF16NP

_CACHE = {}

_ONE_SET = "natural_log_exp_and_others"


def _patch_act_tables():
    from concourse import hw_specs, bacc as bacc_mod
    if getattr(bacc_mod, "_act_tables_patched", False):
        return
    orig = hw_specs.get_activation_tables

    def patched(arch):
        t = orig(arch)
        return {name: (funcs if name == _ONE_SET else set())
                for name, funcs in t.items()}

    bacc_mod.get_activation_tables = patched
    bacc_mod._act_tables_patched = True


def _build():
    _patch_act_tables()
    nc = bacc.Bacc(None, target_bir_lowering=False, debug=False)

    fT_ext = nc.declare_dram_parameter("fT", [D, B], BF16, isOutput=False)
    wT_ext = nc.declare_dram_parameter("wT", [D, CS], BF16, isOutput=False)
    fnat_ext = nc.declare_dram_parameter("fnat", [B, D], BF16, isOutput=False)
    wtgt_ext = nc.declare_dram_parameter("wtgt", [B, D], BF16, isOutput=False)
    out_ext = nc.declare_dram_parameter("out", [1, 1], F32, isOutput=True)

    ccA_in = nc.dram_tensor("ccA_in", [128, 2 * NB], F32)
    ccA_out = nc.dram_tensor("ccA_out", [128, 2 * NB], F32,
                             addr_space="Shared")
    ccB_in = nc.dram_tensor("ccB_in", [128, 2 * NB], F32)
    ccB_out = nc.dram_tensor("ccB_out", [128, 2 * NB], F32,
                             addr_space="Shared")

    with tile.TileContext(nc) as tc:
        with (
            tc.tile_pool(name="persist", bufs=1) as pp,
            tc.tile_pool(name="stream", bufs=4) as sp,
        ):
            # ---- persistent SBUF tiles ----
            wt3 = pp.tile([128, NK, CS], BF16)     # raw wT (bf16)
            whats = [pp.tile([128, NK, 512], MMDT, tag=f"what{i}",
                             name=f"what{i}")
                     for i in range(NCC)]          # normalized wT, per chunk
            ft3 = pp.tile([128, NK, B], BF16)      # raw fT (bf16)
            fhat3 = pp.tile([128, NK, B], MMDT)    # 64 * normalized fT
            ones_bf = pp.tile([128, 1], BF16)
            inv_bf = pp.tile([128, 1], BF16)       # 1/4096: folds 64^2 in
            ones_f32 = pp.tile([128, 1], F32)
            ones_row = pp.tile([1, 128], BF16)
            sums4 = pp.tile([128, 4 * NB], F32)    # exp sums per (cc-pair, b)
            rawdot = pp.tile([128, NB], F32)
            ssf = pp.tile([128, NB], F32)
            wn2 = pp.tile([128, NB], F32)

            # ---- DMA the matmul operands in, split per k-chunk ----
            wTr = wT_ext[:].rearrange("(k p) c -> p k c", p=128)
            fTr = fT_ext[:].rearrange("(k p) b -> p k b", p=128)
            for k in range(NK):
                nc.sync.dma_start(wt3[:, k, :], wTr[:, k, :])
            for k in range(NK):
                nc.sync.dma_start(ft3[:, k, :], fTr[:, k, :])

            nc.vector.memset(ones_bf[:], 1.0)
            nc.vector.memset(inv_bf[:], 1.0 / 4096.0)
            nc.vector.memset(ones_f32[:], 1.0)
            nc.vector.memset(ones_row[:], 1.0)

            def norm_chunk(psml, src3, col0, lhs_const, dst_slices,
                           sq_engine):
                """rowsum -> 1/sqrt via exp(-ln/2) -> bcast -> scale."""
                ps = psml.tile([1, 512], F32, tag="rowsum", name="ps")
                for k in range(NK):
                    sq = sp.tile([128, 512], BF16, tag="sqt", name="sq")
                    sq_engine.tensor_mul(sq[:], src3[:, k, col0:col0 + 512],
                                         src3[:, k, col0:col0 + 512])
                    nc.tensor.matmul(ps[:], lhs_const[:], sq[:],
                                     start=(k == 0), stop=(k == NK - 1))
                lrow = sp.tile([1, 512], F32, tag="lrow", name="lrow")
                nc.scalar.activation(lrow[:], ps[:], AF.Ln)
                rnr = sp.tile([1, 512], BF16, tag="rnr", name="rnr")
                nc.scalar.activation(rnr[:], lrow[:], AF.Exp, scale=-0.5)
                pb = psml.tile([128, 512], F32, tag="bcast", name="pb")
                nc.tensor.matmul(pb[:], ones_row[:], rnr[:],
                                 start=True, stop=True)
                bc = sp.tile([128, 512], BF16, tag="bc", name="bc")
                nc.vector.tensor_copy(bc[:], pb[:])
                last = None
                for k, dst in dst_slices:
                    last = nc.vector.tensor_mul(
                        dst, src3[:, k, col0:col0 + 512], bc[:])
                return last

            psml_cm = tc.tile_pool(name="psmall", bufs=2, space="PSUM")
            psml = psml_cm.__enter__()
            pmain_cm = tc.tile_pool(name="pmain", bufs=2, space="PSUM")
            pmain = pmain_cm.__enter__()
            # feature norms + 64*normalized features (main needs these first)
            for n in range(NBC):
                norm_chunk(
                    psml, ft3, 512 * n, inv_bf,
                    [(k, fhat3[:, k, bass.ts(n, 512)]) for k in range(NK)],
                    nc.vector)
            # weight-col norms + normalized weight, per 512-chunk, paired
            # with the main-loop sweep that consumes them
            last_exp = None
            last_chunk = None
            for q in range(4):  # cc-pair index
                for i in range(2):
                    n = 2 * q + i
                    last_chunk = norm_chunk(
                        psml, wt3, 512 * n, ones_bf,
                        [(k, whats[n][:, k, :]) for k in range(NK)],
                        nc.gpsimd)
                for b in range(NB):
                    zp = pmain.tile([128, 1024], F32, tag="z", name="zp")
                    for c2 in range(2):
                        cc = 2 * q + c2
                        if USE_FP8:
                            for j in range(NK // 2):
                                nc.tensor.matmul(
                                    zp[:, bass.ts(c2, 512)],
                                    fhat3[:, 2 * j:2 * j + 2,
                                          bass.ts(b, 128)],
                                    whats[cc][:, 2 * j:2 * j + 2, :],
                                    start=(j == 0), stop=(j == 1),
                                    perf_mode=mybir.MatmulPerfMode.DoubleRow)
                        else:
                            for k in range(NK):
                                nc.tensor.matmul(
                                    zp[:, bass.ts(c2, 512)],
                                    fhat3[:, k, bass.ts(b, 128)],
                                    whats[cc][:, k, :],
                                    start=(k == 0), stop=(k == NK - 1))
                    last_exp = nc.scalar.activation(
                        zp[:], zp[:], AF.Exp,
                        accum_out=sums4[:, q * NB + b: q * NB + b + 1])
                if q == 1:
                    # first-half AllReduce hides under the second half
                    nc.sync.dma_start(ccA_in[:], sums4[:, 0:2 * NB])
                    nc.gpsimd.collective_compute(
                        "AllReduce", ALU.add,
                        replica_groups=[list(range(NCORES))],
                        ins=[ccA_in[:].opt()],
                        outs=[ccA_out[:].opt()],
                    )

            nc.sync.dma_start(ccB_in[:], sums4[:, 2 * NB:4 * NB])
            nc.gpsimd.collective_compute(
                "AllReduce", ALU.add,
                replica_groups=[list(range(NCORES))],
                ins=[ccB_in[:].opt()],
                outs=[ccB_out[:].opt()],
            )
            fullsumA = pp.tile([128, 2 * NB], F32)
            nc.sync.dma_start(fullsumA[:], ccA_out[:])
            fullsumB = pp.tile([128, 2 * NB], F32)
            nc.sync.dma_start(fullsumB[:], ccB_out[:])

            # ---- target path (concurrent with main loop; GpSimd + DVE) ----
            for t in range(NB):
                fn = sp.tile([128, D], BF16, tag="fnat", name="fn")
                nc.sync.dma_start(fn[:], fnat_ext[bass.ts(t, 128), :])
                wg = sp.tile([128, D], BF16, tag="wtgtn", name="wg")
                nc.sync.dma_start(wg[:], wtgt_ext[bass.ts(t, 128), :])
                prod = sp.tile([128, D], BF16, tag="prod", name="prod")
                tm = nc.gpsimd.tensor_mul(prod[:], fn[:], wg[:])
                if t == 0:
                    add_dep_helper(tm.ins, last_chunk.ins,
                                   reason="tgt path after norm prep")
                nc.vector.reduce_sum(rawdot[:, t:t + 1], prod[:],
                                     axis=mybir.AxisListType.X)
                sq1 = sp.tile([128, D], BF16, tag="prod", name="sq1")
                nc.gpsimd.tensor_mul(sq1[:], fn[:], fn[:])
                nc.vector.reduce_sum(ssf[:, t:t + 1], sq1[:],
                                     axis=mybir.AxisListType.X)
                sq2 = sp.tile([128, D], BF16, tag="prod", name="sq2")
                nc.gpsimd.tensor_mul(sq2[:], wg[:], wg[:])
                nc.vector.reduce_sum(wn2[:, t:t + 1], sq2[:],
                                     axis=mybir.AxisListType.X)

            # ---- combine: ACT ops gated behind the last main-loop Exp ----
            m2 = pp.tile([128, NB], F32)
            nc.vector.tensor_mul(m2[:], ssf[:], wn2[:])
            lm2 = pp.tile([128, NB], F32)
            ln_gate = nc.scalar.activation(lm2[:], m2[:], AF.Ln)
            add_dep_helper(ln_gate.ins, last_exp.ins,
                           reason="keep combine ACT ops after main-loop exps")
            rboth = pp.tile([128, NB], F32)
            nc.scalar.activation(rboth[:], lm2[:], AF.Exp, scale=-0.5)
            tgt = pp.tile([128, NB], F32)
            nc.vector.tensor_mul(tgt[:], rawdot[:], rboth[:])
            exptgt = pp.tile([128, NB], F32)
            nc.scalar.activation(exptgt[:], tgt[:], AF.Exp, scale=S)
            tclip = pp.tile([128, NB], F32)
            nc.vector.tensor_scalar(
                tclip[:], tgt[:], -1.0 + EPS, 1.0 - EPS,
                op0=ALU.max, op1=ALU.min)
            om = pp.tile([128, NB], F32)
            nc.vector.tensor_mul(om[:], tclip[:], tclip[:])
            nc.vector.tensor_scalar(om[:], om[:], -1.0, 1.0,
                                    op0=ALU.mult, op1=ALU.add)
            # sqrt(om) = exp(0.5*ln(om))
            lom = pp.tile([128, NB], F32)
            nc.scalar.activation(lom[:], om[:], AF.Ln)
            snt = pp.tile([128, NB], F32)
            nc.scalar.activation(snt[:], lom[:], AF.Exp, scale=0.5)
            num = pp.tile([128, NB], F32)
            nc.vector.tensor_scalar_mul(num[:], tclip[:], S * COSM)
            snts = pp.tile([128, NB], F32)
            nc.vector.tensor_scalar_mul(snts[:], snt[:], S * SINM)
            nc.vector.tensor_sub(num[:], num[:], snts[:])
            expnum = pp.tile([128, NB], F32)
            nc.scalar.activation(expnum[:], num[:], AF.Exp)

            # ---- final combine (identical on every core) ----
            fullsum = pp.tile([128, NB], F32)
            nc.vector.tensor_add(fullsum[:], fullsumA[:, 0:NB],
                                 fullsumA[:, NB:2 * NB])
            nc.vector.tensor_add(fullsum[:], fullsum[:],
                                 fullsumB[:, 0:NB])
            nc.vector.tensor_add(fullsum[:], fullsum[:],
                                 fullsumB[:, NB:2 * NB])
            denom = pp.tile([128, NB], F32)
            nc.vector.tensor_add(denom[:], expnum[:], fullsum[:])
            nc.vector.tensor_sub(denom[:], denom[:], exptgt[:])
            logd = pp.tile([128, NB], F32)
            nc.scalar.activation(logd[:], denom[:], AF.Ln)
            lvals = pp.tile([128, NB], F32)
            nc.vector.tensor_sub(lvals[:], num[:], logd[:])
            lred = pp.tile([128, 1], F32)
            nc.vector.reduce_sum(lred[:], lvals[:], axis=mybir.AxisListType.X)
            pfin = psml.tile([1, 1], F32, tag="rowsum", name="pfin")
            nc.tensor.matmul(pfin[:], ones_f32[:], lred[:],
                             start=True, stop=True)
            outv = pp.tile([1, 1], F32)
            nc.scalar.mul(outv[:], pfin[:], -1.0 / float(B))
            nc.sync.dma_start(out_ext[:], outv[:])
            pmain_cm.__exit__(None, None, None)
            psml_cm.__exit__(None, None, None)

    nc.compile()
    return nc


def _prep_inputs(features, y_true, weight):
    features = np.asarray(features, dtype=np.float32)
    weight = np.asarray(weight, dtype=np.float32)
    y = np.asarray(y_true).astype(np.int64)

    fT = features.T.astype(BF16NP, order="C")          # [D, B]
    fnat = features.astype(BF16NP)                     # [B, D] bf16
    wtgt = weight[y].astype(BF16NP)                    # [B, D] bf16

    in_maps = []
    for i in range(NCORES):
        shard = weight[i * CS:(i + 1) * CS]            # [CS, D]
        wT = shard.T.astype(BF16NP, order="C")         # [D, CS]
        in_maps.append({"fT": fT, "wT": wT, "fnat": fnat, "wtgt": wtgt})
    return in_maps


def _run(features, y_true, weight, trace=False, **run_kwargs):
    if "nc" not in _CACHE:
        _CACHE["nc"] = _build()
    nc = _CACHE["nc"]
    in_maps = _prep_inputs(features, y_true, weight)
    res = run_bass_kernel_spmd(
        nc, in_maps, core_ids=list(range(NCORES)), trace=trace, **run_kwargs)
    out = np.asarray(res.results[0]["out"], dtype=np.float32)
    return np.float32(out.reshape(-1)[0]), res


def kernel(features, y_true, weight):
    val, _ = _run(features, y_true, weight, trace=False)
    return np.asarray(val, dtype=np.float32)
